# revision 16
# baseline (speedup 1.0000x reference)
"""Trainium2 Bass kernel for per-pixel kernel application (KPN-style ApplyKernel).

y[c,h,w] = sum_{ii,jj} xpad[c, h+ii, w+jj] * k[ii*11+jj, h, w]

Strategy (8 NeuronCores, data-parallel over W strips of 160 cols):
  - Host: pad x and build per-partition row-window slabs (6 rows + 10-row halo
    per partition, shifts live in the free dim) in bf16, two column-alignment
    variants so every tap's DVE read stays 4-byte aligned (keeps
    tensor_tensor in its 2x bf16 mode). k is re-laid-out host-side to
    row-block-major [120, 121, 960] with even-column taps first, so each DMA
    descriptor moves a large contiguous per-partition chunk (>=30 KB) and the
    odd-variant slab load hides behind the first ~66 taps of compute.
  - Device, per group of taps: one SWDGE DMA of the k group (f32->bf16 cast
    in flight); per tap one VectorE tensor_tensor multiply (bf16 2x mode) and
    6 TensorE identity-matmuls accumulating into 6 PSUM banks
    (3 channels x 512/448-col chunks).
  - Epilogue: ScalarE+VectorE evacuate PSUM -> SBUF per channel, overlapped
    per-channel DMAs out, host-side reshape of y.
"""

import sys

if "/opt/trn_rl_repo" not in sys.path:
    sys.path.insert(0, "/opt/trn_rl_repo")

import numpy as np
import ml_dtypes

import concourse.mybir as mybir
from concourse import bacc
from concourse.tile import TileContext, add_dep_helper
from concourse.bass_utils import run_bass_kernel_spmd

KS = 11
HALF = 5
H, W, C = 720, 1280, 3
NCORES = 8
WS = W // NCORES            # 160 cols per core
RPP = 6                     # output rows per partition
NP = H // RPP               # 120 partitions used
ROWS_PP = RPP + 2 * HALF    # 16 rows stored per partition
WPAD = WS + 2 * HALF        # 170 cols stored per partition
SLABF = C * ROWS_PP * WPAD  # 8160 bf16 per partition per variant
NTAPS = KS * KS             # 121
FD = RPP * WS               # 960 elements per channel per tap
PFD = C * FD                # 2880 product elements per tap

# tap order: even-jj taps first (only need slab variant 0), then odd-jj
TAP_PERM = ([t for t in range(NTAPS) if (t % KS) % 2 == 0]
            + [t for t in range(NTAPS) if (t % KS) % 2 == 1])
# k DMA group sizes over the permuted order (66 even + 55 odd taps)
GROUPS = [2] + [8] * 8 + [8] * 6 + [7]
assert sum(GROUPS) == NTAPS and sum(GROUPS[:9]) == 66

BF16 = ml_dtypes.bfloat16

_CACHE = {}


def _build_nc(taps=NTAPS):
    nc = bacc.Bacc("TRN2", target_bir_lowering=False, debug=False)
    k_d = nc.dram_tensor("k", [NP, NTAPS, FD], mybir.dt.float32, kind="ExternalInput")
    xs_d = nc.dram_tensor("xs", [2, 128, SLABF], mybir.dt.bfloat16, kind="ExternalInput")
    id_d = nc.dram_tensor("ident", [NP, NP], mybir.dt.bfloat16, kind="ExternalInput")
    y_d = nc.dram_tensor("y", [NP, PFD], mybir.dt.float32, kind="ExternalOutput")

    with TileContext(nc) as tc:
        with tc.tile_pool(name="const", bufs=1) as const_pool, \
             tc.tile_pool(name="kbf", bufs=3) as kb_pool, \
             tc.tile_pool(name="prod", bufs=6) as prod_pool, \
             tc.tile_pool(name="out", bufs=1) as out_pool, \
             tc.tile_pool(name="psum", bufs=1, space="PSUM") as psum_pool:

            slab0 = const_pool.tile([128, SLABF], mybir.dt.bfloat16)
            slab1 = const_pool.tile([128, SLABF], mybir.dt.bfloat16)
            ident = const_pool.tile([NP, NP], mybir.dt.bfloat16)
            nc.sync.dma_start(ident[:], id_d.ap())
            nc.sync.dma_start(slab0[:], xs_d.ap()[0])
            slab_views = [
                s[:].rearrange("p (c r w) -> p c r w", c=C, r=ROWS_PP, w=WPAD)
                for s in (slab0, slab1)]

            accs = []
            for c in range(C):
                a0 = psum_pool.tile([NP, 512], mybir.dt.float32, name=f"acc{c}0")
                a1 = psum_pool.tile([NP, 448], mybir.dt.float32, name=f"acc{c}1")
                accs.append((a0, a1))

            gi0 = 0
            groups = []
            for ng in GROUPS:
                if gi0 >= taps:
                    break
                groups.append((gi0, min(ng, taps - gi0)))
                gi0 += ng
            for gidx, (gi0, ng) in enumerate(groups):
                kb = kb_pool.tile([NP, ng * FD], mybir.dt.bfloat16, name="kb")
                kdma = nc.gpsimd.dma_start(
                    kb[:].rearrange("p (t f) -> p t f", t=ng),
                    k_d.ap()[:, gi0:gi0 + ng, :])
                if gidx == 1:
                    # odd-column-variant slab (first needed at tap 67): start
                    # it only after group 1's k DMA so the pipeline-fill
                    # loads (slab0 + k group 0) get full DMA bandwidth.
                    s1dma = nc.sync.dma_start(slab1[:], xs_d.ap()[1])
                    add_dep_helper(kdma.ins, s1dma.ins, sync=True,
                                   reason="delay slab1 load past pipeline fill")


                for dt_ in range(ng):
                    gi = gi0 + dt_
                    t = TAP_PERM[gi]
                    ii, jj = divmod(t, KS)
                    v = jj & 1
                    jj2 = jj - v
                    xs_op = slab_views[v][0:NP, :, ii:ii + RPP, jj2:jj2 + WS]
                    k_op = (kb[0:NP, dt_ * FD:(dt_ + 1) * FD]
                            .rearrange("p (r w) -> p r w", r=RPP)
                            .unsqueeze(1).broadcast_to([NP, C, RPP, WS]))
                    prod = prod_pool.tile([NP, PFD], mybir.dt.bfloat16, name="prod")
                    prod_view = prod[0:NP, :].rearrange(
                        "p (c r w) -> p c r w", c=C, r=RPP, w=WS)
                    nc.vector.tensor_tensor(prod_view, xs_op, k_op,
                                            mybir.AluOpType.mult)
                    first = (gi == 0)
                    last = (gi == taps - 1)
                    for c in range(C):
                        nc.tensor.matmul(accs[c][0][:], ident[:],
                                         prod[0:NP, c * FD:c * FD + 512],
                                         start=first, stop=last)
                        nc.tensor.matmul(accs[c][1][:], ident[:],
                                         prod[0:NP, c * FD + 512:(c + 1) * FD],
                                         start=first, stop=last)

            yst = out_pool.tile([NP, PFD], mybir.dt.float32)
            for c in range(C):
                nc.scalar.copy(yst[0:NP, c * FD:c * FD + 512], accs[c][0][:])
                nc.vector.tensor_copy(yst[0:NP, c * FD + 512:(c + 1) * FD],
                                      accs[c][1][:])
                nc.sync.dma_start(y_d.ap()[:, c * FD:(c + 1) * FD],
                                  yst[0:NP, c * FD:(c + 1) * FD])

    nc.compile()
    return nc


def get_nc(taps=NTAPS):
    if taps not in _CACHE:
        _CACHE[taps] = _build_nc(taps)
    return _CACHE[taps]


def _prep_inputs(x, k, padding, padding_value):
    """Host-side prep: pad x, build bf16 slabs + per-core shards."""
    x = np.asarray(x, dtype=np.float32)
    k = np.asarray(k, dtype=np.float32)
    pad = bool(int(np.asarray(padding)))
    pv = float(np.asarray(padding_value))

    if pad:
        assert x.shape == (1, C, H, W), x.shape
        xp = np.full((C, H + 2 * HALF, W + 2 * HALF), pv, dtype=np.float32)
        xp[:, HALF:HALF + H, HALF:HALF + W] = x[0]
    else:
        assert x.shape == (1, C, H + 2 * HALF, W + 2 * HALF), x.shape
        xp = np.ascontiguousarray(x[0])

    assert k.shape == (1, NTAPS, H, W), k.shape
    # row-block-major + tap-permuted k: kt[p, t, (r w)] = k[perm[t], RPP*p+r, w]
    kt = np.ascontiguousarray(
        k[0][TAP_PERM].reshape(NTAPS, NP, RPP * W).transpose(1, 0, 2))

    rows_idx = RPP * np.arange(128)[:, None] + np.arange(ROWS_PP)[None, :]
    ident = np.eye(NP, dtype=BF16)
    in_maps = []
    for ci in range(NCORES):
        w0 = WS * ci
        strip = xp[:, :, w0:w0 + WPAD]                     # [C, 730, 170]
        spad = np.zeros((C, RPP * 127 + ROWS_PP, WPAD + 1), dtype=np.float32)
        spad[:, :H + 2 * HALF, :WPAD] = strip
        xs = np.empty((2, 128, SLABF), dtype=BF16)
        for v in (0, 1):
            sv = spad[:, :, v:v + WPAD]                    # [C, 778, 170]
            win = sv[:, rows_idx, :]                       # [C, 128, 16, 170]
            xs[v] = win.transpose(1, 0, 2, 3).reshape(128, SLABF).astype(BF16)
        kshard = np.ascontiguousarray(
            kt.reshape(NP, NTAPS, RPP, W)[:, :, :, w0:w0 + WS]
            .reshape(NP, NTAPS, FD))
        in_maps.append({"k": kshard, "xs": xs, "ident": ident})
    return in_maps


def _assemble_y(results):
    """results[ci]["y"] is [120, 2880]; reassemble to [1, C, H, W]."""
    y = np.empty((C, H, W), dtype=np.float32)
    for ci in range(NCORES):
        blk = results[ci]["y"].reshape(NP, C, RPP, WS)     # [p, c, r, w]
        y[:, :, WS * ci:WS * (ci + 1)] = blk.transpose(1, 0, 2, 3).reshape(C, H, WS)
    return y[None]


def kernel(x, k, padding, padding_value):
    in_maps = _prep_inputs(x, k, padding, padding_value)
    nc = get_nc()
    res = run_bass_kernel_spmd(nc, in_maps, core_ids=list(range(NCORES)))
    return _assemble_y(res.results).astype(np.float32)


# revision 19
# speedup vs baseline: 1.0114x; 1.0114x over previous
"""Trainium2 Bass kernel for per-pixel kernel application (KPN-style ApplyKernel).

y[c,h,w] = sum_{ii,jj} xpad[c, h+ii, w+jj] * k[ii*11+jj, h, w]

Strategy (8 NeuronCores, data-parallel over W strips of 160 cols):
  - Host: pad x and build per-partition row-window slabs (6 rows + 10-row halo
    per partition, shifts live in the free dim) in bf16, two column-alignment
    variants so every tap's DVE read stays 4-byte aligned (keeps
    tensor_tensor in its 2x bf16 mode). k is re-laid-out host-side to
    row-block-major [120, 121, 960] with even-column taps first, so each DMA
    descriptor moves a large contiguous per-partition chunk (>=30 KB) and the
    odd-variant slab load hides behind the first ~66 taps of compute.
  - Device, per group of taps: one SWDGE DMA of the k group (f32->bf16 cast
    in flight); per tap one VectorE tensor_tensor multiply (bf16 2x mode) and
    6 TensorE identity-matmuls accumulating into 6 PSUM banks
    (3 channels x 512/448-col chunks).
  - Epilogue: ScalarE+VectorE evacuate PSUM -> SBUF per channel, overlapped
    per-channel DMAs out, host-side reshape of y.
"""

import sys

if "/opt/trn_rl_repo" not in sys.path:
    sys.path.insert(0, "/opt/trn_rl_repo")

import numpy as np
import ml_dtypes

import concourse.mybir as mybir
from concourse import bacc
from concourse.tile import TileContext
from concourse.bass_utils import run_bass_kernel_spmd

KS = 11
HALF = 5
H, W, C = 720, 1280, 3
NCORES = 8
WS = W // NCORES            # 160 cols per core
RPP = 6                     # output rows per partition
NP = H // RPP               # 120 partitions used
ROWS_PP = RPP + 2 * HALF    # 16 rows stored per partition
WPAD = WS + 2 * HALF        # 170 cols stored per partition
SLABF = C * ROWS_PP * WPAD  # 8160 bf16 per partition per variant
NTAPS = KS * KS             # 121
FD = RPP * WS               # 960 elements per channel per tap
PFD = C * FD                # 2880 product elements per tap

# tap order: even-jj taps first (only need slab variant 0), then odd-jj
TAP_PERM = ([t for t in range(NTAPS) if (t % KS) % 2 == 0]
            + [t for t in range(NTAPS) if (t % KS) % 2 == 1])
# k DMA group sizes over the permuted order (66 even + 55 odd taps)
GROUPS = [2] + [8] * 8 + [8] * 6 + [7]
assert sum(GROUPS) == NTAPS and sum(GROUPS[:9]) == 66

BF16 = ml_dtypes.bfloat16

_CACHE = {}


def _build_nc(taps=NTAPS):
    nc = bacc.Bacc("TRN2", target_bir_lowering=False, debug=False)
    k_d = nc.dram_tensor("k", [NP, NTAPS, FD], mybir.dt.float32, kind="ExternalInput")
    xs_d = nc.dram_tensor("xs", [2, 128, SLABF], mybir.dt.bfloat16, kind="ExternalInput")
    id_d = nc.dram_tensor("ident", [NP, NP], mybir.dt.bfloat16, kind="ExternalInput")
    y_d = nc.dram_tensor("y", [NP, PFD], mybir.dt.float32, kind="ExternalOutput")

    with TileContext(nc) as tc:
        with tc.tile_pool(name="const", bufs=1) as const_pool, \
             tc.tile_pool(name="kbf", bufs=3) as kb_pool, \
             tc.tile_pool(name="prod", bufs=6) as prod_pool, \
             tc.tile_pool(name="out", bufs=1) as out_pool, \
             tc.tile_pool(name="psum", bufs=1, space="PSUM") as psum_pool:

            slab0 = const_pool.tile([128, SLABF], mybir.dt.bfloat16)
            slab1 = const_pool.tile([128, SLABF], mybir.dt.bfloat16)
            ident = const_pool.tile([NP, NP], mybir.dt.bfloat16)
            nc.sync.dma_start(ident[:], id_d.ap())
            nc.sync.dma_start(slab0[:], xs_d.ap()[0])
            nc.sync.dma_start(slab1[:], xs_d.ap()[1])
            slab_views = [
                s[:].rearrange("p (c r w) -> p c r w", c=C, r=ROWS_PP, w=WPAD)
                for s in (slab0, slab1)]

            accs = []
            for c in range(C):
                a0 = psum_pool.tile([NP, 512], mybir.dt.float32, name=f"acc{c}0")
                a1 = psum_pool.tile([NP, 448], mybir.dt.float32, name=f"acc{c}1")
                accs.append((a0, a1))

            gi0 = 0
            groups = []
            for ng in GROUPS:
                if gi0 >= taps:
                    break
                groups.append((gi0, min(ng, taps - gi0)))
                gi0 += ng
            for gidx, (gi0, ng) in enumerate(groups):
                kb = kb_pool.tile([NP, ng * FD], mybir.dt.bfloat16, name="kb")
                nc.gpsimd.dma_start(
                    kb[:].rearrange("p (t f) -> p t f", t=ng),
                    k_d.ap()[:, gi0:gi0 + ng, :])


                for dt_ in range(ng):
                    gi = gi0 + dt_
                    t = TAP_PERM[gi]
                    ii, jj = divmod(t, KS)
                    v = jj & 1
                    jj2 = jj - v
                    xs_op = slab_views[v][0:NP, :, ii:ii + RPP, jj2:jj2 + WS]
                    k_op = (kb[0:NP, dt_ * FD:(dt_ + 1) * FD]
                            .rearrange("p (r w) -> p r w", r=RPP)
                            .unsqueeze(1).broadcast_to([NP, C, RPP, WS]))
                    prod = prod_pool.tile([NP, PFD], mybir.dt.bfloat16, name="prod")
                    prod_view = prod[0:NP, :].rearrange(
                        "p (c r w) -> p c r w", c=C, r=RPP, w=WS)
                    nc.vector.tensor_tensor(prod_view, xs_op, k_op,
                                            mybir.AluOpType.mult)
                    first = (gi == 0)
                    last = (gi == taps - 1)
                    for c in range(C):
                        nc.tensor.matmul(accs[c][0][:], ident[:],
                                         prod[0:NP, c * FD:c * FD + 512],
                                         start=first, stop=last)
                        nc.tensor.matmul(accs[c][1][:], ident[:],
                                         prod[0:NP, c * FD + 512:(c + 1) * FD],
                                         start=first, stop=last)

            yst = out_pool.tile([NP, PFD], mybir.dt.float32)
            for c in range(C):
                nc.scalar.copy(yst[0:NP, c * FD:c * FD + 512], accs[c][0][:])
                nc.vector.tensor_copy(yst[0:NP, c * FD + 512:(c + 1) * FD],
                                      accs[c][1][:])
                nc.sync.dma_start(y_d.ap()[:, c * FD:(c + 1) * FD],
                                  yst[0:NP, c * FD:(c + 1) * FD])

    nc.compile()
    return nc


def get_nc(taps=NTAPS):
    if taps not in _CACHE:
        _CACHE[taps] = _build_nc(taps)
    return _CACHE[taps]


def _prep_inputs(x, k, padding, padding_value):
    """Host-side prep: pad x, build bf16 slabs + per-core shards."""
    x = np.asarray(x, dtype=np.float32)
    k = np.asarray(k, dtype=np.float32)
    pad = bool(int(np.asarray(padding)))
    pv = float(np.asarray(padding_value))

    if pad:
        assert x.shape == (1, C, H, W), x.shape
        xp = np.full((C, H + 2 * HALF, W + 2 * HALF), pv, dtype=np.float32)
        xp[:, HALF:HALF + H, HALF:HALF + W] = x[0]
    else:
        assert x.shape == (1, C, H + 2 * HALF, W + 2 * HALF), x.shape
        xp = np.ascontiguousarray(x[0])

    assert k.shape == (1, NTAPS, H, W), k.shape
    # row-block-major + tap-permuted k: kt[p, t, (r w)] = k[perm[t], RPP*p+r, w]
    kt = np.ascontiguousarray(
        k[0][TAP_PERM].reshape(NTAPS, NP, RPP * W).transpose(1, 0, 2))

    rows_idx = RPP * np.arange(128)[:, None] + np.arange(ROWS_PP)[None, :]
    ident = np.eye(NP, dtype=BF16)
    in_maps = []
    for ci in range(NCORES):
        w0 = WS * ci
        strip = xp[:, :, w0:w0 + WPAD]                     # [C, 730, 170]
        spad = np.zeros((C, RPP * 127 + ROWS_PP, WPAD + 1), dtype=np.float32)
        spad[:, :H + 2 * HALF, :WPAD] = strip
        xs = np.empty((2, 128, SLABF), dtype=BF16)
        for v in (0, 1):
            sv = spad[:, :, v:v + WPAD]                    # [C, 778, 170]
            win = sv[:, rows_idx, :]                       # [C, 128, 16, 170]
            xs[v] = win.transpose(1, 0, 2, 3).reshape(128, SLABF).astype(BF16)
        kshard = np.ascontiguousarray(
            kt.reshape(NP, NTAPS, RPP, W)[:, :, :, w0:w0 + WS]
            .reshape(NP, NTAPS, FD))
        in_maps.append({"k": kshard, "xs": xs, "ident": ident})
    return in_maps


def _assemble_y(results):
    """results[ci]["y"] is [120, 2880]; reassemble to [1, C, H, W]."""
    y = np.empty((C, H, W), dtype=np.float32)
    for ci in range(NCORES):
        blk = results[ci]["y"].reshape(NP, C, RPP, WS)     # [p, c, r, w]
        y[:, :, WS * ci:WS * (ci + 1)] = blk.transpose(1, 0, 2, 3).reshape(C, H, WS)
    return y[None]


def kernel(x, k, padding, padding_value):
    in_maps = _prep_inputs(x, k, padding, padding_value)
    nc = get_nc()
    res = run_bass_kernel_spmd(nc, in_maps, core_ids=list(range(NCORES)))
    return _assemble_y(res.results).astype(np.float32)
